# revision 16
# baseline (speedup 1.0000x reference)
"""Trainium2 Bass kernel for nn_ChromaticResonance — v5.

Math (per batch row, complex wave z, D=512, 7 depths):
  p* = ch @ {C+H1(+I), H2, H3, H5}
  y  = pW + 0.25|p2|^2 (re only) + (1/9)|p3|^2 p3 + (1/25) p5^5 |p5|^-4.8
  t  = tanh(y*s + b);  ch' = fd_d * t;  out += w_d * ch'

v5 on top of v4 (1.626ms, PE 96% busy at the bf16 roofline):
  - H5 matmuls in fp8 e4m3 with perf_mode=DoubleRow: K=512 contraction in
    2 MMs of K=256 instead of 4 of K=128 (~11% less PE time). Weights are
    pre-scaled x8 and sigma x16 on the host/device; the 1/128 comes out in
    the p5 evacuation. Verified in a numpy prototype: rel err 1.30e-2
    (threshold 2e-2).
  - tanh's scale/bias fold into the pW evacuation (ACT applies
    f(scale*x+bias) with per-partition APs for free), so tanh becomes ONE
    plain full-tile op; all chain tiles switch to the sigma layout
    [128, KT, 2, nb] so tanh/cast/out ops are contiguous 1:1.
  - chain reordered so only acc+=bw -> tanh -> sigma sit after the last
    psum evacuation (short tail).
  Custom DVE ops (CUBE3 for h3, QUINTIC for h5 = w^5 with the magnitude
  correction pre-folded into w via one bitcast-log2 Exp) as in v3/v4.
  GpSimd stays EMPTY: any GpSimd op grabs the DVE-shared SBUF port pair
  and blocks concurrent 2-src DVE ops for its whole duration.
"""

import numpy as np
import ml_dtypes

import concourse.bass as bass
import concourse.mybir as mybir
import concourse.tile as tile
from concourse import bass_utils
from concourse.bacc import Bacc
import concourse.dve_ops as dve_ops
from concourse.dve_spec import Spec, Src0, Src1, C0, C1, sq, lower, _has_src1
from concourse.dve_uop import DveOpSpec

F32 = mybir.dt.float32
BF16 = mybir.dt.bfloat16
FP8 = mybir.dt.float8e4
I32 = mybir.dt.int32
AF = mybir.ActivationFunctionType
OP = mybir.AluOpType
DR = mybir.MatmulPerfMode.DoubleRow

B, D, DEPTH = 32768, 512, 7
N_CORES = 8
BS = B // N_CORES
NB = 512
KT = D // 128

LN2 = float(np.log(2.0))
SIGMA0 = 0.0430
EXP_SCALE15 = float(-0.48 * LN2 * 2.0 ** -23)
_wv = np.exp(-np.linspace(0.0, 2.0, DEPTH))
WV = (_wv / _wv.sum()).astype(np.float64)  # output depth weights (compile-time)

QA = float(5.0 + 2.0 * np.sqrt(5.0))  # x^4-10x^2y^2+5y^4 = (x^2-QA y^2)(x^2-QB y^2)
QB = float(5.0 - 2.0 * np.sqrt(5.0))

W5_SCALE = 8.0    # fp8 weight prescale
SIG_SCALE = 16.0  # fp8 sigma prescale
P5_DESCALE = 1.0 / (W5_SCALE * SIG_SCALE)

# consts columns
C_WF = 0        # 0-6: sigma scale w_d*f_d
C_BIAS = 8      # mixing bias (folded into pW evac)
C_B15 = 9       # 9-15: exp bias for s15, + 0.2*ln(s)
C_BW = 16       # 16-22: pW evac scale kw_d * s
C_SQ2 = 23      # 23-29: p2 evac scale sqrt(k2_d * s)
C_B3 = 30       # 30-36: p3 evac scale cbrt(k3_d * s)
NCONST = 40


def _sha_of(spec, name):
    shas = {}
    for ver in ("v3", "v4"):
        uops = lower(spec, ver=ver)
        shas[ver] = DveOpSpec(name=name, opcode=None, uops=uops,
                              rd1_en=_has_src1(spec)).sha(ver)
    return shas


def _register_custom_ops():
    if "R2_SUM_SQ_ANT" in dve_ops._SUB_OPCODE_FOR_NAME:
        return (dve_ops._R2_SUM_SQ_ANT, dve_ops._CUBE3_ANT, dve_ops._QUINTIC5_ANT)
    r2 = dve_ops.DveOp(
        "R2_SUM_SQ_ANT",
        Spec(body=sq(Src0) + sq(Src1),
             reference=lambda in0, in1, s0, s1, imm2: (
                 in0.astype(np.float32) ** 2 + in1.astype(np.float32) ** 2
             ).astype(np.float32)),
        subdim=False,
        uops_sha={"v3": "cd4bd6e1c27efd14", "v4": "121e32d8332f5047"},
    )
    from concourse.dve_spec import C2
    cube3_spec = Spec(
        body=Src0 * (sq(Src0) + sq(Src1)) * C0,
        reference=lambda in0, in1, s0, s1, imm2: (
            in0.astype(np.float32)
            * (in0.astype(np.float32) ** 2 + in1.astype(np.float32) ** 2)
            * s0
        ).astype(np.float32))
    cube3 = dve_ops.DveOp("CUBE3_ANT", cube3_spec, subdim=False,
                          uops_sha=_sha_of(cube3_spec, "CUBE3_ANT"))

    def _quintic_ref(in0, in1, s0, s1, imm2):
        x = in0.astype(np.float32)
        y = in1.astype(np.float32)
        return (x * ((x * x - s0 * y * y) * (x * x - s1 * y * y))).astype(np.float32)

    _sx, _sy = sq(Src0), sq(Src1)
    quintic_spec = Spec(body=Src0 * ((_sx - _sy * C0) * (_sx - _sy * C1)),
                        reference=_quintic_ref)
    quintic = dve_ops.DveOp("QUINTIC5_ANT", quintic_spec, subdim=False,
                            uops_sha=_sha_of(quintic_spec, "QUINTIC5_ANT"))
    for op in (r2, cube3, quintic):
        dve_ops.OPS.append(op)
        dve_ops.CUSTOM_DVE_SPECS[op.name] = op.spec
        dve_ops._SUB_OPCODE_FOR_NAME[op.name] = (
            dve_ops._CUSTOM_DVE_ROW_BASE + len(dve_ops.OPS) - 1)
    dve_ops._R2_SUM_SQ_ANT = r2
    dve_ops._CUBE3_ANT = cube3
    dve_ops._QUINTIC5_ANT = quintic
    return r2, cube3, quintic


def build_program(n_chunks=BS // NB, nb=NB, const_sb=None):
    """const_sb=(s0, b0) when mixing_scale/bias are constant vectors: evac
    scales become float immediates (AP scale/bias costs ~+250ns per ACT op)
    and tanh is one plain batched op. const_sb=None keeps the generic
    per-partition AP path."""
    assert n_chunks % 2 == 0
    R2OP, CUBE3, QUINT = _register_custom_ops()
    nc = Bacc()
    bcols = n_chunks * nb

    wre = nc.dram_tensor("wre", [D, bcols], BF16, kind="ExternalInput")
    wim = nc.dram_tensor("wim", [D, bcols], BF16, kind="ExternalInput")
    wre8 = nc.dram_tensor("wre8", [D, bcols], FP8, kind="ExternalInput")
    wim8 = nc.dram_tensor("wim8", [D, bcols], FP8, kind="ExternalInput")
    wmat = nc.dram_tensor("wmat", [4, D, D], BF16, kind="ExternalInput")
    wm5 = nc.dram_tensor("wm5", [D, D], FP8, kind="ExternalInput")
    consts = nc.dram_tensor("consts", [D, NCONST], F32, kind="ExternalInput")
    ore = nc.dram_tensor("ore", [D, bcols], BF16, kind="ExternalOutput")
    oim = nc.dram_tensor("oim", [D, bcols], BF16, kind="ExternalOutput")

    with tile.TileContext(nc) as tc:
        with (
            tc.tile_pool(name="wpool", bufs=1) as wpool,
            tc.tile_pool(name="spool", bufs=1) as spool,   # sigma states + out
            tc.tile_pool(name="ppool", bufs=1, space="PSUM") as ppool,
            tc.tile_pool(name="cpool", bufs=1) as cpool,   # chain scratch
        ):
            # ---- weights + consts (loaded once) ----
            wt = []
            for mi in range(4):
                w = wpool.tile([128, KT, D], BF16, name=f"wt{mi}", tag=f"wt{mi}")
                for k in range(KT):
                    nc.sync.dma_start(out=w[:, k, :],
                                      in_=wmat[mi, k * 128:(k + 1) * 128, :])
                wt.append(w)
            w5 = wpool.tile([128, KT, D], FP8, name="w5", tag="w5")
            for k in range(KT):
                nc.sync.dma_start(out=w5[:, k, :],
                                  in_=wm5[k * 128:(k + 1) * 128, :])
            cons = []
            for m in range(KT):
                c = wpool.tile([128, NCONST], F32, name=f"cons{m}", tag=f"cons{m}")
                nc.sync.dma_start(out=c, in_=consts[m * 128:(m + 1) * 128, :])
                cons.append(c)

            for cp in range(n_chunks // 2):
                sig = {}
                sig8 = {}
                outs = {}
                for sl in range(2):
                    ci = 2 * cp + sl
                    c0 = ci * nb
                    s0t = spool.tile([128, KT, 2, nb], BF16,
                                     name=f"sg{sl}0", tag=f"sg{sl}0")
                    s1t = spool.tile([128, KT, 2, nb], BF16,
                                     name=f"sg{sl}1", tag=f"sg{sl}1")
                    f0t = spool.tile([128, KT, 2, nb], FP8,
                                     name=f"sf{sl}0", tag=f"sf{sl}0")
                    f1t = spool.tile([128, KT, 2, nb], FP8,
                                     name=f"sf{sl}1", tag=f"sf{sl}1")
                    for k in range(KT):
                        nc.sync.dma_start(
                            out=s0t[:, k, 0, :],
                            in_=wre[k * 128:(k + 1) * 128, c0:c0 + nb])
                        nc.sync.dma_start(
                            out=s0t[:, k, 1, :],
                            in_=wim[k * 128:(k + 1) * 128, c0:c0 + nb])
                        nc.sync.dma_start(
                            out=f0t[:, k, 0, :],
                            in_=wre8[k * 128:(k + 1) * 128, c0:c0 + nb])
                        nc.sync.dma_start(
                            out=f0t[:, k, 1, :],
                            in_=wim8[k * 128:(k + 1) * 128, c0:c0 + nb])
                    sig[sl] = [s0t, s1t]
                    sig8[sl] = [f0t, f1t]
                    outs[sl] = spool.tile([128, KT, 2, nb], BF16,
                                          name=f"out{sl}", tag=f"out{sl}", bufs=1)

                for dep in range(DEPTH):
                    w1 = wt[0] if dep == 0 else wt[1]
                    for sl in range(2):
                        scur = sig[sl][dep % 2]
                        snxt = sig[sl][(dep + 1) % 2]
                        f8cur = sig8[sl][dep % 2]
                        f8nxt = sig8[sl][(dep + 1) % 2]
                        out_t = outs[sl]

                        # chain scratch in sigma layout, double-buffered
                        # across the two interleaved chunks. b53 holds the
                        # p5 and p3 evacuations side by side [mat, pl] so one
                        # ACT op evacuates both (they share a 4-bank psum
                        # tile and are plain copies — k3 rides CUBE3's imm).
                        b53 = cpool.tile([128, KT, 2, 2, nb], BF16,
                                         name="b53", tag="b53", bufs=2)
                        sq2 = cpool.tile([128, KT, 2, nb], BF16,
                                         name="sq2", tag="sq2", bufs=2)
                        bw = cpool.tile([128, KT, 2, nb], BF16,
                                        name="bw", tag="bw", bufs=2)
                        acc = cpool.tile([128, KT, 2, nb], BF16,
                                         name="acc", tag="acc", bufs=2)
                        r5 = cpool.tile([128, KT, nb], F32,
                                        name="r5", tag="r5", bufs=1)
                        s15 = cpool.tile([128, KT, nb], BF16,
                                         name="s15", tag="s15", bufs=1)
                        q5 = cpool.tile([128, KT, 2, nb], BF16,
                                        name="q5", tag="q5", bufs=2)

                        for m in range(KT):
                            msl = slice(m * 128, (m + 1) * 128)
                            H = slice(0, nb)
                            I = slice(nb, 2 * nb)

                            def mm_group(pt_ap, lw):
                                for k in range(KT):
                                    for j, hs in enumerate((H, I)):
                                        nc.tensor.matmul(
                                            pt_ap[:, hs], lw[:, k, msl],
                                            scur[:, k, j, :],
                                            start=(k == 0), stop=(k == KT - 1))

                            wprev = 1.0 if dep == 0 else float(WV[dep - 1])
                            # H5 (fp8 DoubleRow, raw scale) + H3 share one
                            # 4-bank psum tile and one evacuation
                            p53t = ppool.tile([128, 4 * nb], F32,
                                              name="p53", tag="p53")
                            for q in range(2):
                                for j, hs in enumerate((H, I)):
                                    nc.tensor.matmul(
                                        p53t[:, hs],
                                        w5[:, 2 * q:2 * q + 2, msl],
                                        f8cur[:, 2 * q:2 * q + 2, j, :],
                                        start=(q == 0), stop=(q == 1),
                                        perf_mode=DR)
                            mm_group(p53t[:, 2 * nb:4 * nb], wt[3])
                            if const_sb is not None:
                                nc.scalar.copy(
                                    b53[:, m, :, :, :],
                                    p53t.rearrange("p (mat two n) -> p mat two n",
                                                   mat=2, two=2))
                            else:
                                nc.scalar.copy(
                                    b53[:, m, 0, :, :],
                                    p53t[:, 0:2 * nb].rearrange(
                                        "p (two n) -> p two n", two=2))
                                nc.scalar.mul(
                                    b53[:, m, 1, :, :],
                                    p53t[:, 2 * nb:4 * nb].rearrange(
                                        "p (two n) -> p two n", two=2),
                                    cons[m][:, C_B3 + dep:C_B3 + dep + 1])

                            p2t = ppool.tile([128, 2 * nb], F32,
                                             name="p2", tag="p2")
                            mm_group(p2t[:, :], wt[2])
                            if const_sb is None:
                                sq2_scale = cons[m][:, C_SQ2 + dep:C_SQ2 + dep + 1]
                            else:
                                sq2_scale = float(
                                    np.sqrt(max(0.25 * const_sb[0], 0.0)) / wprev)
                            nc.scalar.activation(
                                sq2[:, m, :, :],
                                p2t.rearrange("p (two n) -> p two n", two=2),
                                AF.Square, scale=sq2_scale)

                            pWt = ppool.tile([128, 2 * nb], F32,
                                             name="pW", tag="pW")
                            mm_group(pWt[:, :], w1)
                            # pW evac carries the tanh affine: kw*s scale and
                            # the mixing bias, so tanh below is one plain op
                            if const_sb is None:
                                nc.scalar.activation(
                                    bw[:, m, :, :],
                                    pWt.rearrange("p (two n) -> p two n", two=2),
                                    AF.Identity,
                                    scale=cons[m][:, C_BW + dep:C_BW + dep + 1],
                                    bias=cons[m][:, C_BIAS:C_BIAS + 1])
                            elif const_sb[1] != 0.0:
                                nc.scalar.activation(
                                    bw[:, m, :, :],
                                    pWt.rearrange("p (two n) -> p two n", two=2),
                                    AF.Identity,
                                    scale=float(const_sb[0] / wprev),
                                    bias=float(const_sb[1]))
                            else:
                                nc.scalar.mul(
                                    bw[:, m, :, :],
                                    pWt.rearrange("p (two n) -> p two n", two=2),
                                    float(const_sb[0] / wprev))

                        # ---- batched chain ----
                        wprev_d = 1.0 if dep == 0 else float(WV[dep - 1])
                        b5H, b5I = b53[:, :, 0, 0, :], b53[:, :, 0, 1, :]
                        b3H, b3I = b53[:, :, 1, 0, :], b53[:, :, 1, 1, :]
                        accH, accI = acc[:, :, 0, :], acc[:, :, 1, :]
                        if const_sb is not None:
                            k3imm = float(const_sb[0] / 9.0 / wprev_d ** 3)
                        else:
                            k3imm = 1.0

                        nc.vector._custom_dve(R2OP, out=r5[:, :, :],
                                              in0=b5H, in1=b5I)
                        nc.scalar.activation(
                            s15[:, :, :], r5[:, :, :].bitcast(I32), AF.Exp,
                            scale=EXP_SCALE15,
                            bias=cons[0][:, C_B15 + dep:C_B15 + dep + 1])
                        nc.vector._custom_dve(CUBE3, out=accH, in0=b3H, in1=b3I,
                                              s0=k3imm)
                        nc.vector._custom_dve(CUBE3, out=accI, in0=b3I, in1=b3H,
                                              s0=k3imm)
                        nc.vector.tensor_tensor(b5H, b5H, s15[:, :, :],
                                                op=OP.mult)
                        nc.vector.tensor_tensor(b5I, b5I, s15[:, :, :],
                                                op=OP.mult)
                        nc.vector._custom_dve(QUINT, out=q5[:, :, 0, :],
                                              in0=b5H, in1=b5I, s0=QA, s1=QB)
                        nc.vector._custom_dve(QUINT, out=q5[:, :, 1, :],
                                              in0=b5I, in1=b5H, s0=QA, s1=QB)
                        nc.vector.tensor_tensor(acc[:, :, :, :], acc[:, :, :, :],
                                                q5[:, :, :, :], op=OP.add)
                        nc.vector.tensor_tensor(accH, accH, sq2[:, :, 0, :],
                                                op=OP.add)
                        nc.vector.tensor_tensor(accH, accH, sq2[:, :, 1, :],
                                                op=OP.add)
                        # last: the + (kw*s*pW + bias) term — only this add,
                        # tanh and sigma sit after the final evacuation
                        nc.vector.tensor_tensor(acc[:, :, :, :], acc[:, :, :, :],
                                                bw[:, :, :, :], op=OP.add)

                        # per-m tanh + sigma: next depth's k-tile m matmuls
                        # start as soon as sigma[m] lands (short tail)
                        for m in range(KT):
                            nc.scalar.activation(snxt[:, m, :, :],
                                                 acc[:, m, :, :], AF.Tanh)
                            nc.vector.tensor_scalar_mul(
                                snxt[:, m, :, :], snxt[:, m, :, :],
                                cons[m][:, C_WF + dep:C_WF + dep + 1])
                        if dep < DEPTH - 1:
                            # fp8 copy of sigma for next depth's H5, split
                            # across ACT (H plane) and DVE (I plane) so
                            # neither engine eats the whole 4096-elem cast
                            nc.scalar.mul(f8nxt[:, :, 0, :], snxt[:, :, 0, :],
                                          SIG_SCALE)
                            nc.vector.tensor_scalar_mul(
                                f8nxt[:, :, 1, :], snxt[:, :, 1, :], SIG_SCALE)
                        # out accumulation on DVE (GpSimd would block DVE via
                        # the shared SBUF port pair)
                        if dep == 0:
                            nc.vector.tensor_copy(out_t[:, :, :, :],
                                                  snxt[:, :, :, :])
                        else:
                            nc.vector.tensor_tensor(
                                out_t[:, :, :, :], out_t[:, :, :, :],
                                snxt[:, :, :, :], op=OP.add)

                for sl in range(2):
                    ci = 2 * cp + sl
                    c0 = ci * nb
                    for m in range(KT):
                        nc.sync.dma_start(
                            out=ore[m * 128:(m + 1) * 128, c0:c0 + nb],
                            in_=outs[sl][:, m, 0, :])
                        nc.sync.dma_start(
                            out=oim[m * 128:(m + 1) * 128, c0:c0 + nb],
                            in_=outs[sl][:, m, 1, :])
    nc.finalize()
    return nc


def host_prep(coupling_matrix, harmonic_1, harmonic_2, harmonic_3, harmonic_5,
              mixing_scale, mixing_bias):
    damping = (0.1 / (1.0 + np.exp(np.linspace(0.0, 3.0, D)))).astype(np.float64)
    fd = np.stack([np.exp(-damping * dd) for dd in range(DEPTH)])  # [7, D]
    wf = (WV[:, None] * fd).astype(np.float32)                     # [7, D]
    w1_0 = (coupling_matrix + harmonic_1).astype(np.float32)
    w1_r = w1_0 + np.eye(D, dtype=np.float32)
    wmat = np.ascontiguousarray(
        np.stack([w1_0, w1_r, harmonic_2, harmonic_3])
    ).astype(ml_dtypes.bfloat16)
    wm5 = np.ascontiguousarray(harmonic_5.astype(np.float32) * W5_SCALE
                               ).astype(ml_dtypes.float8_e4m3)
    s = mixing_scale.astype(np.float64)
    consts = np.zeros((D, NCONST), np.float32)
    consts[:, 0:DEPTH] = wf.T
    consts[:, C_BIAS] = mixing_bias.astype(np.float32)
    lns = np.log(np.maximum(s, 1e-30))
    for dep in range(DEPTH):
        wprev = 1.0 if dep == 0 else float(WV[dep - 1])
        # -0.28*LN2: p5 is now evacuated raw (128x larger), and
        # s15 must come out 128x smaller: -7ln2 + 0.48*14*ln2/2... exact:
        # bits(128^2 r5) = bits(r5)+14*2^23 -> exp already gives 2^-6.72,
        # need 2^-7 total.
        b15 = (0.48 * LN2 * (127.0 - SIGMA0) + 0.2 * np.log(1.0 / 25.0)
               - 0.08 * np.log(wprev) - 0.28 * LN2)
        consts[:, C_B15 + dep] = (b15 + 0.2 * lns).astype(np.float32)
        consts[:, C_BW + dep] = (s / wprev).astype(np.float32)
        consts[:, C_SQ2 + dep] = np.sqrt(np.maximum(0.25 * s, 0.0)
                                         ).astype(np.float32) / wprev
        consts[:, C_B3 + dep] = np.cbrt((1.0 / 9.0) * s).astype(np.float32) / wprev
    return wmat, wm5, consts


def device_inputs(wave_real, wave_imag, coupling_matrix, harmonic_1,
                  harmonic_2, harmonic_3, harmonic_5, mixing_scale,
                  mixing_bias):
    wmat, wm5, consts = host_prep(coupling_matrix, harmonic_1, harmonic_2,
                                  harmonic_3, harmonic_5, mixing_scale,
                                  mixing_bias)
    wreT = np.asarray(wave_real, np.float32).T
    wimT = np.asarray(wave_imag, np.float32).T
    wre_bf = wreT.astype(ml_dtypes.bfloat16)
    wim_bf = wimT.astype(ml_dtypes.bfloat16)
    wre_f8 = (wreT * SIG_SCALE).astype(ml_dtypes.float8_e4m3)
    wim_f8 = (wimT * SIG_SCALE).astype(ml_dtypes.float8_e4m3)
    in_maps = []
    for c in range(N_CORES):
        sl = slice(c * BS, (c + 1) * BS)
        in_maps.append({
            "wre": np.ascontiguousarray(wre_bf[:, sl]),
            "wim": np.ascontiguousarray(wim_bf[:, sl]),
            "wre8": np.ascontiguousarray(wre_f8[:, sl]),
            "wim8": np.ascontiguousarray(wim_f8[:, sl]),
            "wmat": wmat,
            "wm5": wm5,
            "consts": consts,
        })
    return in_maps


_NC_CACHE = {}


def _const_sb(mixing_scale, mixing_bias):
    s = np.asarray(mixing_scale, np.float32)
    b = np.asarray(mixing_bias, np.float32)
    if np.all(s == s.flat[0]) and np.all(b == b.flat[0]):
        return (float(s.flat[0]), float(b.flat[0]))
    return None


def _get_nc(n_chunks, nb, const_sb=None):
    key = (n_chunks, nb, const_sb)
    if key not in _NC_CACHE:
        _NC_CACHE[key] = build_program(n_chunks, nb, const_sb)
    return _NC_CACHE[key]


def kernel(wave_real, wave_imag, coupling_matrix, harmonic_1, harmonic_2,
           harmonic_3, harmonic_5, mixing_scale, mixing_bias):
    nc = _get_nc(BS // NB, NB, _const_sb(mixing_scale, mixing_bias))
    in_maps = device_inputs(wave_real, wave_imag, coupling_matrix, harmonic_1,
                            harmonic_2, harmonic_3, harmonic_5, mixing_scale,
                            mixing_bias)
    res = bass_utils.run_bass_kernel_spmd(nc, in_maps, core_ids=list(range(N_CORES)))
    out = np.empty((2, B, D), np.float32)
    for c in range(N_CORES):
        sl = slice(c * BS, (c + 1) * BS)
        out[0, sl, :] = res.results[c]["ore"].astype(np.float32).T
        out[1, sl, :] = res.results[c]["oim"].astype(np.float32).T
    return out


# revision 18
# speedup vs baseline: 1.0329x; 1.0329x over previous
"""Trainium2 Bass kernel for nn_ChromaticResonance — v5.

Math (per batch row, complex wave z, D=512, 7 depths):
  p* = ch @ {C+H1(+I), H2, H3, H5}
  y  = pW + 0.25|p2|^2 (re only) + (1/9)|p3|^2 p3 + (1/25) p5^5 |p5|^-4.8
  t  = tanh(y*s + b);  ch' = fd_d * t;  out += w_d * ch'

v5 on top of v4 (1.626ms, PE 96% busy at the bf16 roofline):
  - H5 matmuls in fp8 e4m3 with perf_mode=DoubleRow: K=512 contraction in
    2 MMs of K=256 instead of 4 of K=128 (~11% less PE time). Weights are
    pre-scaled x8 and sigma x16 on the host/device; the 1/128 comes out in
    the p5 evacuation. Verified in a numpy prototype: rel err 1.30e-2
    (threshold 2e-2).
  - tanh's scale/bias fold into the pW evacuation (ACT applies
    f(scale*x+bias) with per-partition APs for free), so tanh becomes ONE
    plain full-tile op; all chain tiles switch to the sigma layout
    [128, KT, 2, nb] so tanh/cast/out ops are contiguous 1:1.
  - chain reordered so only acc+=bw -> tanh -> sigma sit after the last
    psum evacuation (short tail).
  Custom DVE ops (CUBE3 for h3, QUINTIC for h5 = w^5 with the magnitude
  correction pre-folded into w via one bitcast-log2 Exp) as in v3/v4.
  GpSimd stays EMPTY: any GpSimd op grabs the DVE-shared SBUF port pair
  and blocks concurrent 2-src DVE ops for its whole duration.
"""

import numpy as np
import ml_dtypes

import concourse.bass as bass
import concourse.mybir as mybir
import concourse.tile as tile
from concourse import bass_utils
from concourse.bacc import Bacc
import concourse.dve_ops as dve_ops
from concourse.dve_spec import Spec, Src0, Src1, C0, C1, sq, lower, _has_src1
from concourse.dve_uop import DveOpSpec

F32 = mybir.dt.float32
BF16 = mybir.dt.bfloat16
FP8 = mybir.dt.float8e4
I32 = mybir.dt.int32
AF = mybir.ActivationFunctionType
OP = mybir.AluOpType
DR = mybir.MatmulPerfMode.DoubleRow

B, D, DEPTH = 32768, 512, 7
N_CORES = 8
BS = B // N_CORES
NB = 512
KT = D // 128

LN2 = float(np.log(2.0))
SIGMA0 = 0.0430
EXP_SCALE15 = float(-0.48 * LN2 * 2.0 ** -23)
_wv = np.exp(-np.linspace(0.0, 2.0, DEPTH))
WV = (_wv / _wv.sum()).astype(np.float64)  # output depth weights (compile-time)

QA = float(5.0 + 2.0 * np.sqrt(5.0))  # x^4-10x^2y^2+5y^4 = (x^2-QA y^2)(x^2-QB y^2)
QB = float(5.0 - 2.0 * np.sqrt(5.0))

W5_SCALE = 8.0    # fp8 weight prescale
SIG_SCALE = 16.0  # fp8 sigma prescale
P5_DESCALE = 1.0 / (W5_SCALE * SIG_SCALE)

# consts columns
C_WF = 0        # 0-6: sigma scale w_d*f_d
C_BIAS = 8      # mixing bias (folded into pW evac)
C_B15 = 9       # 9-15: exp bias for s15, + 0.2*ln(s)
C_BW = 16       # 16-22: pW evac scale kw_d * s
C_SQ2 = 23      # 23-29: p2 evac scale sqrt(k2_d * s)
C_B3 = 30       # 30-36: p3 evac scale cbrt(k3_d * s)
NCONST = 40


def _sha_of(spec, name):
    shas = {}
    for ver in ("v3", "v4"):
        uops = lower(spec, ver=ver)
        shas[ver] = DveOpSpec(name=name, opcode=None, uops=uops,
                              rd1_en=_has_src1(spec)).sha(ver)
    return shas


def _register_custom_ops():
    if "R2_SUM_SQ_ANT" in dve_ops._SUB_OPCODE_FOR_NAME:
        return (dve_ops._R2_SUM_SQ_ANT, dve_ops._CUBE3_ANT, dve_ops._QUINTIC5_ANT)
    r2 = dve_ops.DveOp(
        "R2_SUM_SQ_ANT",
        Spec(body=sq(Src0) + sq(Src1),
             reference=lambda in0, in1, s0, s1, imm2: (
                 in0.astype(np.float32) ** 2 + in1.astype(np.float32) ** 2
             ).astype(np.float32)),
        subdim=False,
        uops_sha={"v3": "cd4bd6e1c27efd14", "v4": "121e32d8332f5047"},
    )
    cube3_spec = Spec(
        body=Src0 * (sq(Src0) + sq(Src1)),
        reference=lambda in0, in1, s0, s1, imm2: (
            in0.astype(np.float32)
            * (in0.astype(np.float32) ** 2 + in1.astype(np.float32) ** 2)
        ).astype(np.float32))
    cube3 = dve_ops.DveOp("CUBE3_ANT", cube3_spec, subdim=False,
                          uops_sha=_sha_of(cube3_spec, "CUBE3_ANT"))

    def _quintic_ref(in0, in1, s0, s1, imm2):
        x = in0.astype(np.float32)
        y = in1.astype(np.float32)
        return (x * ((x * x - s0 * y * y) * (x * x - s1 * y * y))).astype(np.float32)

    _sx, _sy = sq(Src0), sq(Src1)
    quintic_spec = Spec(body=Src0 * ((_sx - _sy * C0) * (_sx - _sy * C1)),
                        reference=_quintic_ref)
    quintic = dve_ops.DveOp("QUINTIC5_ANT", quintic_spec, subdim=False,
                            uops_sha=_sha_of(quintic_spec, "QUINTIC5_ANT"))
    for op in (r2, cube3, quintic):
        dve_ops.OPS.append(op)
        dve_ops.CUSTOM_DVE_SPECS[op.name] = op.spec
        dve_ops._SUB_OPCODE_FOR_NAME[op.name] = (
            dve_ops._CUSTOM_DVE_ROW_BASE + len(dve_ops.OPS) - 1)
    dve_ops._R2_SUM_SQ_ANT = r2
    dve_ops._CUBE3_ANT = cube3
    dve_ops._QUINTIC5_ANT = quintic
    return r2, cube3, quintic


def build_program(n_chunks=BS // NB, nb=NB, const_sb=None):
    """const_sb=(s0, b0) when mixing_scale/bias are constant vectors: evac
    scales become float immediates (AP scale/bias costs ~+250ns per ACT op)
    and tanh is one plain batched op. const_sb=None keeps the generic
    per-partition AP path."""
    assert n_chunks % 2 == 0
    R2OP, CUBE3, QUINT = _register_custom_ops()
    nc = Bacc()
    bcols = n_chunks * nb

    wre = nc.dram_tensor("wre", [D, bcols], BF16, kind="ExternalInput")
    wim = nc.dram_tensor("wim", [D, bcols], BF16, kind="ExternalInput")
    wre8 = nc.dram_tensor("wre8", [D, bcols], FP8, kind="ExternalInput")
    wim8 = nc.dram_tensor("wim8", [D, bcols], FP8, kind="ExternalInput")
    wmat = nc.dram_tensor("wmat", [4, D, D], BF16, kind="ExternalInput")
    wm5 = nc.dram_tensor("wm5", [D, D], FP8, kind="ExternalInput")
    consts = nc.dram_tensor("consts", [D, NCONST], F32, kind="ExternalInput")
    ore = nc.dram_tensor("ore", [D, bcols], BF16, kind="ExternalOutput")
    oim = nc.dram_tensor("oim", [D, bcols], BF16, kind="ExternalOutput")

    with tile.TileContext(nc) as tc:
        with (
            tc.tile_pool(name="wpool", bufs=1) as wpool,
            tc.tile_pool(name="spool", bufs=1) as spool,   # sigma states + out
            tc.tile_pool(name="ppool", bufs=1, space="PSUM") as ppool,
            tc.tile_pool(name="cpool", bufs=1) as cpool,   # chain scratch
        ):
            # ---- weights + consts (loaded once) ----
            wt = []
            for mi in range(4):
                w = wpool.tile([128, KT, D], BF16, name=f"wt{mi}", tag=f"wt{mi}")
                for k in range(KT):
                    nc.sync.dma_start(out=w[:, k, :],
                                      in_=wmat[mi, k * 128:(k + 1) * 128, :])
                wt.append(w)
            w5 = wpool.tile([128, KT, D], FP8, name="w5", tag="w5")
            for k in range(KT):
                nc.sync.dma_start(out=w5[:, k, :],
                                  in_=wm5[k * 128:(k + 1) * 128, :])
            cons = []
            for m in range(KT):
                c = wpool.tile([128, NCONST], F32, name=f"cons{m}", tag=f"cons{m}")
                nc.sync.dma_start(out=c, in_=consts[m * 128:(m + 1) * 128, :])
                cons.append(c)

            for cp in range(n_chunks // 2):
                sig = {}
                sig8 = {}
                outs = {}
                for sl in range(2):
                    ci = 2 * cp + sl
                    c0 = ci * nb
                    s0t = spool.tile([128, KT, 2, nb], BF16,
                                     name=f"sg{sl}0", tag=f"sg{sl}0")
                    s1t = spool.tile([128, KT, 2, nb], BF16,
                                     name=f"sg{sl}1", tag=f"sg{sl}1")
                    f0t = spool.tile([128, KT, 2, nb], FP8,
                                     name=f"sf{sl}0", tag=f"sf{sl}0")
                    f1t = spool.tile([128, KT, 2, nb], FP8,
                                     name=f"sf{sl}1", tag=f"sf{sl}1")
                    for k in range(KT):
                        nc.sync.dma_start(
                            out=s0t[:, k, 0, :],
                            in_=wre[k * 128:(k + 1) * 128, c0:c0 + nb])
                        nc.sync.dma_start(
                            out=s0t[:, k, 1, :],
                            in_=wim[k * 128:(k + 1) * 128, c0:c0 + nb])
                        nc.sync.dma_start(
                            out=f0t[:, k, 0, :],
                            in_=wre8[k * 128:(k + 1) * 128, c0:c0 + nb])
                        nc.sync.dma_start(
                            out=f0t[:, k, 1, :],
                            in_=wim8[k * 128:(k + 1) * 128, c0:c0 + nb])
                    sig[sl] = [s0t, s1t]
                    sig8[sl] = [f0t, f1t]
                    outs[sl] = spool.tile([128, KT, 2, nb], BF16,
                                          name=f"out{sl}", tag=f"out{sl}", bufs=1)

                for dep in range(DEPTH):
                    w1 = wt[0] if dep == 0 else wt[1]
                    for sl in range(2):
                        scur = sig[sl][dep % 2]
                        snxt = sig[sl][(dep + 1) % 2]
                        f8cur = sig8[sl][dep % 2]
                        f8nxt = sig8[sl][(dep + 1) % 2]
                        out_t = outs[sl]

                        # chain scratch in sigma layout [128, KT, 2, nb],
                        # double-buffered across the two interleaved chunks
                        b3 = cpool.tile([128, KT, 2, nb], BF16,
                                        name="b3", tag="b3q5", bufs=2)
                        sq2 = cpool.tile([128, KT, 2, nb], BF16,
                                         name="sq2", tag="sq2", bufs=2)
                        b5 = cpool.tile([128, KT, 2, nb], BF16,
                                        name="b5", tag="b5", bufs=2)
                        bw = cpool.tile([128, KT, 2, nb], BF16,
                                        name="bw", tag="bw", bufs=2)
                        acc = cpool.tile([128, KT, 2, nb], BF16,
                                         name="acc", tag="acc", bufs=2)
                        r5 = cpool.tile([128, KT, nb], F32,
                                        name="r5", tag="r5", bufs=2)
                        s15 = cpool.tile([128, KT, nb], BF16,
                                         name="s15", tag="s15", bufs=2)
                        q5 = cpool.tile([128, KT, 2, nb], BF16,
                                        name="q5", tag="b3q5", bufs=2)

                        H = slice(0, nb)
                        I = slice(nb, 2 * nb)
                        wprev = 1.0 if dep == 0 else float(WV[dep - 1])

                        def mm_group(pt_ap, lw, msl):
                            for k in range(KT):
                                for j, hs in enumerate((H, I)):
                                    nc.tensor.matmul(
                                        pt_ap[:, hs], lw[:, k, msl],
                                        scur[:, k, j, :],
                                        start=(k == 0), stop=(k == KT - 1))

                        # pass 1: H5 (fp8 DoubleRow) + H3 for ALL m first —
                        # their chain (R2/Exp/CUBE3/QUINT) is the long pole,
                        # so its inputs land ~10us earlier and the sigma for
                        # the next depth completes with slack instead of
                        # stalling the PE 2-3.4us every depth. Each tag's
                        # evacuation hides behind the other matrix's group.
                        for m in range(KT):
                            msl = slice(m * 128, (m + 1) * 128)
                            p5t = ppool.tile([128, 2 * nb], F32,
                                             name="p5", tag="p5")
                            for q in range(2):
                                for j, hs in enumerate((H, I)):
                                    nc.tensor.matmul(
                                        p5t[:, hs],
                                        w5[:, 2 * q:2 * q + 2, msl],
                                        f8cur[:, 2 * q:2 * q + 2, j, :],
                                        start=(q == 0), stop=(q == 1),
                                        perf_mode=DR)
                            nc.scalar.mul(
                                b5[:, m, :, :],
                                p5t.rearrange("p (two n) -> p two n", two=2),
                                P5_DESCALE)

                            p3t = ppool.tile([128, 2 * nb], F32,
                                             name="p3", tag="p3")
                            mm_group(p3t[:, :], wt[3], msl)
                            if const_sb is None:
                                b3_scale = cons[m][:, C_B3 + dep:C_B3 + dep + 1]
                            else:
                                b3_scale = float(
                                    np.cbrt(const_sb[0] / 9.0) / wprev)
                            nc.scalar.mul(
                                b3[:, m, :, :],
                                p3t.rearrange("p (two n) -> p two n", two=2),
                                b3_scale)

                        # pass 2: H2 + W1
                        for m in range(KT):
                            msl = slice(m * 128, (m + 1) * 128)
                            p2t = ppool.tile([128, 2 * nb], F32,
                                             name="p2", tag="p2")
                            mm_group(p2t[:, :], wt[2], msl)
                            if const_sb is None:
                                sq2_scale = cons[m][:, C_SQ2 + dep:C_SQ2 + dep + 1]
                            else:
                                sq2_scale = float(
                                    np.sqrt(max(0.25 * const_sb[0], 0.0)) / wprev)
                            nc.scalar.activation(
                                sq2[:, m, :, :],
                                p2t.rearrange("p (two n) -> p two n", two=2),
                                AF.Square, scale=sq2_scale)

                            pWt = ppool.tile([128, 2 * nb], F32,
                                             name="pW", tag="pW")
                            mm_group(pWt[:, :], w1, msl)
                            # pW evac carries the tanh affine: kw*s scale and
                            # the mixing bias, so tanh below is one plain op
                            if const_sb is None:
                                nc.scalar.activation(
                                    bw[:, m, :, :],
                                    pWt.rearrange("p (two n) -> p two n", two=2),
                                    AF.Identity,
                                    scale=cons[m][:, C_BW + dep:C_BW + dep + 1],
                                    bias=cons[m][:, C_BIAS:C_BIAS + 1])
                            elif const_sb[1] != 0.0:
                                nc.scalar.activation(
                                    bw[:, m, :, :],
                                    pWt.rearrange("p (two n) -> p two n", two=2),
                                    AF.Identity,
                                    scale=float(const_sb[0] / wprev),
                                    bias=float(const_sb[1]))
                            else:
                                nc.scalar.mul(
                                    bw[:, m, :, :],
                                    pWt.rearrange("p (two n) -> p two n", two=2),
                                    float(const_sb[0] / wprev))

                        # ---- batched chain ----
                        b3H, b3I = b3[:, :, 0, :], b3[:, :, 1, :]
                        accH, accI = acc[:, :, 0, :], acc[:, :, 1, :]
                        b5H, b5I = b5[:, :, 0, :], b5[:, :, 1, :]

                        nc.vector._custom_dve(R2OP, out=r5[:, :, :],
                                              in0=b5H, in1=b5I)
                        nc.scalar.activation(
                            s15[:, :, :], r5[:, :, :].bitcast(I32), AF.Exp,
                            scale=EXP_SCALE15,
                            bias=cons[0][:, C_B15 + dep:C_B15 + dep + 1])
                        nc.vector._custom_dve(CUBE3, out=accH, in0=b3H, in1=b3I)
                        nc.vector._custom_dve(CUBE3, out=accI, in0=b3I, in1=b3H)
                        nc.vector.tensor_tensor(b5H, b5H, s15[:, :, :],
                                                op=OP.mult)
                        nc.vector.tensor_tensor(b5I, b5I, s15[:, :, :],
                                                op=OP.mult)
                        nc.vector._custom_dve(QUINT, out=q5[:, :, 0, :],
                                              in0=b5H, in1=b5I, s0=QA, s1=QB)
                        nc.vector._custom_dve(QUINT, out=q5[:, :, 1, :],
                                              in0=b5I, in1=b5H, s0=QA, s1=QB)
                        nc.vector.tensor_tensor(acc[:, :, :, :], acc[:, :, :, :],
                                                q5[:, :, :, :], op=OP.add)
                        nc.vector.tensor_tensor(accH, accH, sq2[:, :, 0, :],
                                                op=OP.add)
                        nc.vector.tensor_tensor(accH, accH, sq2[:, :, 1, :],
                                                op=OP.add)
                        # last: the + (kw*s*pW + bias) term — only this add,
                        # tanh and sigma sit after the final evacuation
                        nc.vector.tensor_tensor(acc[:, :, :, :], acc[:, :, :, :],
                                                bw[:, :, :, :], op=OP.add)

                        nc.scalar.activation(snxt[:, :, :, :], acc[:, :, :, :],
                                             AF.Tanh)
                        for m in range(KT):
                            nc.vector.tensor_scalar_mul(
                                snxt[:, m, :, :], snxt[:, m, :, :],
                                cons[m][:, C_WF + dep:C_WF + dep + 1])
                        if dep < DEPTH - 1:
                            # fp8 copy of sigma for next depth's H5, split
                            # across ACT (H plane) and DVE (I plane) so
                            # neither engine eats the whole 4096-elem cast
                            nc.scalar.mul(f8nxt[:, :, 0, :], snxt[:, :, 0, :],
                                          SIG_SCALE)
                            nc.vector.tensor_scalar_mul(
                                f8nxt[:, :, 1, :], snxt[:, :, 1, :], SIG_SCALE)
                        # out accumulation on DVE (GpSimd would block DVE via
                        # the shared SBUF port pair)
                        if dep == 0:
                            nc.vector.tensor_copy(out_t[:, :, :, :],
                                                  snxt[:, :, :, :])
                        else:
                            nc.vector.tensor_tensor(
                                out_t[:, :, :, :], out_t[:, :, :, :],
                                snxt[:, :, :, :], op=OP.add)

                for sl in range(2):
                    ci = 2 * cp + sl
                    c0 = ci * nb
                    for m in range(KT):
                        nc.sync.dma_start(
                            out=ore[m * 128:(m + 1) * 128, c0:c0 + nb],
                            in_=outs[sl][:, m, 0, :])
                        nc.sync.dma_start(
                            out=oim[m * 128:(m + 1) * 128, c0:c0 + nb],
                            in_=outs[sl][:, m, 1, :])
    nc.finalize()
    return nc


def host_prep(coupling_matrix, harmonic_1, harmonic_2, harmonic_3, harmonic_5,
              mixing_scale, mixing_bias):
    damping = (0.1 / (1.0 + np.exp(np.linspace(0.0, 3.0, D)))).astype(np.float64)
    fd = np.stack([np.exp(-damping * dd) for dd in range(DEPTH)])  # [7, D]
    wf = (WV[:, None] * fd).astype(np.float32)                     # [7, D]
    w1_0 = (coupling_matrix + harmonic_1).astype(np.float32)
    w1_r = w1_0 + np.eye(D, dtype=np.float32)
    wmat = np.ascontiguousarray(
        np.stack([w1_0, w1_r, harmonic_2, harmonic_3])
    ).astype(ml_dtypes.bfloat16)
    wm5 = np.ascontiguousarray(harmonic_5.astype(np.float32) * W5_SCALE
                               ).astype(ml_dtypes.float8_e4m3)
    s = mixing_scale.astype(np.float64)
    consts = np.zeros((D, NCONST), np.float32)
    consts[:, 0:DEPTH] = wf.T
    consts[:, C_BIAS] = mixing_bias.astype(np.float32)
    lns = np.log(np.maximum(s, 1e-30))
    for dep in range(DEPTH):
        wprev = 1.0 if dep == 0 else float(WV[dep - 1])
        b15 = (0.48 * LN2 * (127.0 - SIGMA0) + 0.2 * np.log(1.0 / 25.0)
               - 0.08 * np.log(wprev))
        consts[:, C_B15 + dep] = (b15 + 0.2 * lns).astype(np.float32)
        consts[:, C_BW + dep] = (s / wprev).astype(np.float32)
        consts[:, C_SQ2 + dep] = np.sqrt(np.maximum(0.25 * s, 0.0)
                                         ).astype(np.float32) / wprev
        consts[:, C_B3 + dep] = np.cbrt((1.0 / 9.0) * s).astype(np.float32) / wprev
    return wmat, wm5, consts


def device_inputs(wave_real, wave_imag, coupling_matrix, harmonic_1,
                  harmonic_2, harmonic_3, harmonic_5, mixing_scale,
                  mixing_bias):
    wmat, wm5, consts = host_prep(coupling_matrix, harmonic_1, harmonic_2,
                                  harmonic_3, harmonic_5, mixing_scale,
                                  mixing_bias)
    wreT = np.asarray(wave_real, np.float32).T
    wimT = np.asarray(wave_imag, np.float32).T
    wre_bf = wreT.astype(ml_dtypes.bfloat16)
    wim_bf = wimT.astype(ml_dtypes.bfloat16)
    wre_f8 = (wreT * SIG_SCALE).astype(ml_dtypes.float8_e4m3)
    wim_f8 = (wimT * SIG_SCALE).astype(ml_dtypes.float8_e4m3)
    in_maps = []
    for c in range(N_CORES):
        sl = slice(c * BS, (c + 1) * BS)
        in_maps.append({
            "wre": np.ascontiguousarray(wre_bf[:, sl]),
            "wim": np.ascontiguousarray(wim_bf[:, sl]),
            "wre8": np.ascontiguousarray(wre_f8[:, sl]),
            "wim8": np.ascontiguousarray(wim_f8[:, sl]),
            "wmat": wmat,
            "wm5": wm5,
            "consts": consts,
        })
    return in_maps


_NC_CACHE = {}


def _const_sb(mixing_scale, mixing_bias):
    s = np.asarray(mixing_scale, np.float32)
    b = np.asarray(mixing_bias, np.float32)
    if np.all(s == s.flat[0]) and np.all(b == b.flat[0]):
        return (float(s.flat[0]), float(b.flat[0]))
    return None


def _get_nc(n_chunks, nb, const_sb=None):
    key = (n_chunks, nb, const_sb)
    if key not in _NC_CACHE:
        _NC_CACHE[key] = build_program(n_chunks, nb, const_sb)
    return _NC_CACHE[key]


def kernel(wave_real, wave_imag, coupling_matrix, harmonic_1, harmonic_2,
           harmonic_3, harmonic_5, mixing_scale, mixing_bias):
    nc = _get_nc(BS // NB, NB, _const_sb(mixing_scale, mixing_bias))
    in_maps = device_inputs(wave_real, wave_imag, coupling_matrix, harmonic_1,
                            harmonic_2, harmonic_3, harmonic_5, mixing_scale,
                            mixing_bias)
    res = bass_utils.run_bass_kernel_spmd(nc, in_maps, core_ids=list(range(N_CORES)))
    out = np.empty((2, B, D), np.float32)
    for c in range(N_CORES):
        sl = slice(c * BS, (c + 1) * BS)
        out[0, sl, :] = res.results[c]["ore"].astype(np.float32).T
        out[1, sl, :] = res.results[c]["oim"].astype(np.float32).T
    return out


# revision 22
# speedup vs baseline: 1.0496x; 1.0162x over previous
"""Trainium2 Bass kernel for nn_ChromaticResonance — v5.

Math (per batch row, complex wave z, D=512, 7 depths):
  p* = ch @ {C+H1(+I), H2, H3, H5}
  y  = pW + 0.25|p2|^2 (re only) + (1/9)|p3|^2 p3 + (1/25) p5^5 |p5|^-4.8
  t  = tanh(y*s + b);  ch' = fd_d * t;  out += w_d * ch'

v5 on top of v4 (1.626ms, PE 96% busy at the bf16 roofline):
  - H5 matmuls in fp8 e4m3 with perf_mode=DoubleRow: K=512 contraction in
    2 MMs of K=256 instead of 4 of K=128 (~11% less PE time). Weights are
    pre-scaled x8 and sigma x16 on the host/device; the 1/128 comes out in
    the p5 evacuation. Verified in a numpy prototype: rel err 1.30e-2
    (threshold 2e-2).
  - tanh's scale/bias fold into the pW evacuation (ACT applies
    f(scale*x+bias) with per-partition APs for free), so tanh becomes ONE
    plain full-tile op; all chain tiles switch to the sigma layout
    [128, KT, 2, nb] so tanh/cast/out ops are contiguous 1:1.
  - chain reordered so only acc+=bw -> tanh -> sigma sit after the last
    psum evacuation (short tail).
  Custom DVE ops (CUBE3 for h3, QUINTIC for h5 = w^5 with the magnitude
  correction pre-folded into w via one bitcast-log2 Exp) as in v3/v4.
  GpSimd stays EMPTY: any GpSimd op grabs the DVE-shared SBUF port pair
  and blocks concurrent 2-src DVE ops for its whole duration.
"""

import numpy as np
import ml_dtypes

import concourse.bass as bass
import concourse.mybir as mybir
import concourse.tile as tile
from concourse import bass_utils
from concourse.bacc import Bacc
import concourse.dve_ops as dve_ops
from concourse.dve_spec import Spec, Src0, Src1, C0, C1, sq, lower, _has_src1
from concourse.dve_uop import DveOpSpec

F32 = mybir.dt.float32
BF16 = mybir.dt.bfloat16
FP8 = mybir.dt.float8e4
I32 = mybir.dt.int32
AF = mybir.ActivationFunctionType
OP = mybir.AluOpType
DR = mybir.MatmulPerfMode.DoubleRow

B, D, DEPTH = 32768, 512, 7
N_CORES = 8
BS = B // N_CORES
NB = 512
KT = D // 128

LN2 = float(np.log(2.0))
SIGMA0 = 0.0430
EXP_SCALE15 = float(-0.48 * LN2 * 2.0 ** -23)
_wv = np.exp(-np.linspace(0.0, 2.0, DEPTH))
WV = (_wv / _wv.sum()).astype(np.float64)  # output depth weights (compile-time)

QA = float(5.0 + 2.0 * np.sqrt(5.0))  # x^4-10x^2y^2+5y^4 = (x^2-QA y^2)(x^2-QB y^2)
QB = float(5.0 - 2.0 * np.sqrt(5.0))

W5_SCALE = 8.0    # fp8 weight prescale
SIG_SCALE = 16.0  # fp8 sigma prescale
P5_DESCALE = 1.0 / (W5_SCALE * SIG_SCALE)

# consts columns
C_WF = 0        # 0-6: sigma scale w_d*f_d
C_BIAS = 8      # mixing bias (folded into pW evac)
C_B15 = 9       # 9-15: exp bias for s15, + 0.2*ln(s)
C_BW = 16       # 16-22: pW evac scale kw_d * s
C_SQ2 = 23      # 23-29: p2 evac scale sqrt(k2_d * s)
C_B3 = 30       # 30-36: p3 evac scale cbrt(k3_d * s)
NCONST = 40


def _sha_of(spec, name):
    shas = {}
    for ver in ("v3", "v4"):
        uops = lower(spec, ver=ver)
        shas[ver] = DveOpSpec(name=name, opcode=None, uops=uops,
                              rd1_en=_has_src1(spec)).sha(ver)
    return shas


def _register_custom_ops():
    if "R2_SUM_SQ_ANT" in dve_ops._SUB_OPCODE_FOR_NAME:
        return (dve_ops._R2_SUM_SQ_ANT, dve_ops._CUBE3_ANT, dve_ops._QUINTIC5_ANT)
    r2 = dve_ops.DveOp(
        "R2_SUM_SQ_ANT",
        Spec(body=sq(Src0) + sq(Src1),
             reference=lambda in0, in1, s0, s1, imm2: (
                 in0.astype(np.float32) ** 2 + in1.astype(np.float32) ** 2
             ).astype(np.float32)),
        subdim=False,
        uops_sha={"v3": "cd4bd6e1c27efd14", "v4": "121e32d8332f5047"},
    )
    cube3_spec = Spec(
        body=Src0 * (sq(Src0) + sq(Src1)),
        reference=lambda in0, in1, s0, s1, imm2: (
            in0.astype(np.float32)
            * (in0.astype(np.float32) ** 2 + in1.astype(np.float32) ** 2)
        ).astype(np.float32))
    cube3 = dve_ops.DveOp("CUBE3_ANT", cube3_spec, subdim=False,
                          uops_sha=_sha_of(cube3_spec, "CUBE3_ANT"))

    def _quintic_ref(in0, in1, s0, s1, imm2):
        x = in0.astype(np.float32)
        y = in1.astype(np.float32)
        return (x * ((x * x - s0 * y * y) * (x * x - s1 * y * y))).astype(np.float32)

    _sx, _sy = sq(Src0), sq(Src1)
    quintic_spec = Spec(body=Src0 * ((_sx - _sy * C0) * (_sx - _sy * C1)),
                        reference=_quintic_ref)
    quintic = dve_ops.DveOp("QUINTIC5_ANT", quintic_spec, subdim=False,
                            uops_sha=_sha_of(quintic_spec, "QUINTIC5_ANT"))
    for op in (r2, cube3, quintic):
        dve_ops.OPS.append(op)
        dve_ops.CUSTOM_DVE_SPECS[op.name] = op.spec
        dve_ops._SUB_OPCODE_FOR_NAME[op.name] = (
            dve_ops._CUSTOM_DVE_ROW_BASE + len(dve_ops.OPS) - 1)
    dve_ops._R2_SUM_SQ_ANT = r2
    dve_ops._CUBE3_ANT = cube3
    dve_ops._QUINTIC5_ANT = quintic
    return r2, cube3, quintic


def build_program(n_chunks=BS // NB, nb=NB, const_sb=None):
    """const_sb=(s0, b0) when mixing_scale/bias are constant vectors: evac
    scales become float immediates (AP scale/bias costs ~+250ns per ACT op)
    and tanh is one plain batched op. const_sb=None keeps the generic
    per-partition AP path."""
    assert n_chunks % 2 == 0
    R2OP, CUBE3, QUINT = _register_custom_ops()
    nc = Bacc()
    bcols = n_chunks * nb

    wre = nc.dram_tensor("wre", [D, bcols], BF16, kind="ExternalInput")
    wim = nc.dram_tensor("wim", [D, bcols], BF16, kind="ExternalInput")
    wre8 = nc.dram_tensor("wre8", [D, bcols], FP8, kind="ExternalInput")
    wim8 = nc.dram_tensor("wim8", [D, bcols], FP8, kind="ExternalInput")
    wmat = nc.dram_tensor("wmat", [4, D, D], BF16, kind="ExternalInput")
    wm5 = nc.dram_tensor("wm5", [D, D], FP8, kind="ExternalInput")
    consts = nc.dram_tensor("consts", [D, NCONST], F32, kind="ExternalInput")
    ore = nc.dram_tensor("ore", [D, bcols], BF16, kind="ExternalOutput")
    oim = nc.dram_tensor("oim", [D, bcols], BF16, kind="ExternalOutput")

    with tile.TileContext(nc) as tc:
        with (
            tc.tile_pool(name="wpool", bufs=1) as wpool,
            tc.tile_pool(name="spool", bufs=1) as spool,   # sigma states + out
            tc.tile_pool(name="ppool", bufs=1, space="PSUM") as ppool,
            tc.tile_pool(name="cpool", bufs=1) as cpool,   # chain scratch
        ):
            # ---- weights + consts (loaded once) ----
            w5 = wpool.tile([128, KT, D], FP8, name="w5", tag="w5")
            for k in range(KT):
                nc.sync.dma_start(out=w5[:, k, :],
                                  in_=wm5[k * 128:(k + 1) * 128, :])
            wt = []
            for mi in range(4):
                w = wpool.tile([128, KT, D], BF16, name=f"wt{mi}", tag=f"wt{mi}")
                for k in range(KT):
                    nc.sync.dma_start(out=w[:, k, :],
                                      in_=wmat[mi, k * 128:(k + 1) * 128, :])
                wt.append(w)
            cons = []
            for m in range(KT):
                c = wpool.tile([128, NCONST], F32, name=f"cons{m}", tag=f"cons{m}")
                nc.sync.dma_start(out=c, in_=consts[m * 128:(m + 1) * 128, :])
                cons.append(c)

            for cp in range(n_chunks // 2):
                sig = {}
                sig8 = {}
                outs = {}
                for sl in range(2):
                    ci = 2 * cp + sl
                    c0 = ci * nb
                    s0t = spool.tile([128, KT, 2, nb], BF16,
                                     name=f"sg{sl}0", tag=f"sg{sl}0")
                    s1t = spool.tile([128, KT, 2, nb], BF16,
                                     name=f"sg{sl}1", tag=f"sg{sl}1")
                    f0t = spool.tile([128, KT, 2, nb], FP8,
                                     name=f"sf{sl}0", tag=f"sf{sl}0")
                    f1t = spool.tile([128, KT, 2, nb], FP8,
                                     name=f"sf{sl}1", tag=f"sf{sl}1")
                    for k in range(KT):
                        nc.sync.dma_start(
                            out=s0t[:, k, 0, :],
                            in_=wre[k * 128:(k + 1) * 128, c0:c0 + nb])
                        nc.sync.dma_start(
                            out=s0t[:, k, 1, :],
                            in_=wim[k * 128:(k + 1) * 128, c0:c0 + nb])
                        nc.sync.dma_start(
                            out=f0t[:, k, 0, :],
                            in_=wre8[k * 128:(k + 1) * 128, c0:c0 + nb])
                        nc.sync.dma_start(
                            out=f0t[:, k, 1, :],
                            in_=wim8[k * 128:(k + 1) * 128, c0:c0 + nb])
                    sig[sl] = [s0t, s1t]
                    sig8[sl] = [f0t, f1t]
                    outs[sl] = spool.tile([128, KT, 2, nb], BF16,
                                          name=f"out{sl}", tag=f"out{sl}", bufs=1)

                for dep in range(DEPTH):
                    w1 = wt[0] if dep == 0 else wt[1]
                    for sl in range(2):
                        scur = sig[sl][dep % 2]
                        snxt = sig[sl][(dep + 1) % 2]
                        f8cur = sig8[sl][dep % 2]
                        f8nxt = sig8[sl][(dep + 1) % 2]
                        out_t = outs[sl]

                        # chain scratch in sigma layout [128, KT, 2, nb],
                        # double-buffered across the two interleaved chunks
                        b3 = cpool.tile([128, KT, 2, nb], BF16,
                                        name="b3", tag="b3q5", bufs=2)
                        sq2 = cpool.tile([128, KT, 2, nb], BF16,
                                         name="sq2", tag="sq2", bufs=2)
                        b5 = cpool.tile([128, KT, 2, nb], BF16,
                                        name="b5", tag="b5", bufs=2)
                        bw = cpool.tile([128, KT, 2, nb], BF16,
                                        name="bw", tag="bw", bufs=2)
                        acc = cpool.tile([128, KT, 2, nb], BF16,
                                         name="acc", tag="acc", bufs=2)
                        r5 = cpool.tile([128, KT, nb], F32,
                                        name="r5", tag="r5", bufs=2)
                        s15 = cpool.tile([128, KT, nb], BF16,
                                         name="s15", tag="s15", bufs=2)
                        q5 = cpool.tile([128, KT, 2, nb], BF16,
                                        name="q5", tag="b3q5", bufs=2)

                        for m in range(KT):
                            msl = slice(m * 128, (m + 1) * 128)
                            H = slice(0, nb)
                            I = slice(nb, 2 * nb)

                            def mm_group(pt_ap, lw):
                                for k in range(KT):
                                    for j, hs in enumerate((H, I)):
                                        nc.tensor.matmul(
                                            pt_ap[:, hs], lw[:, k, msl],
                                            scur[:, k, j, :],
                                            start=(k == 0), stop=(k == KT - 1))

                            # H5 first (fp8 DoubleRow): longest chain tail
                            p5t = ppool.tile([128, 2 * nb], F32,
                                             name="p5", tag="p5")
                            for q in range(2):
                                for j, hs in enumerate((H, I)):
                                    nc.tensor.matmul(
                                        p5t[:, hs],
                                        w5[:, 2 * q:2 * q + 2, msl],
                                        f8cur[:, 2 * q:2 * q + 2, j, :],
                                        start=(q == 0), stop=(q == 1),
                                        perf_mode=DR)
                            nc.scalar.mul(
                                b5[:, m, :, :],
                                p5t.rearrange("p (two n) -> p two n", two=2),
                                P5_DESCALE)

                            wprev = 1.0 if dep == 0 else float(WV[dep - 1])
                            p3t = ppool.tile([128, 2 * nb], F32,
                                             name="p3", tag="p3")
                            mm_group(p3t[:, :], wt[3])
                            if const_sb is None:
                                b3_scale = cons[m][:, C_B3 + dep:C_B3 + dep + 1]
                            else:
                                b3_scale = float(
                                    np.cbrt(const_sb[0] / 9.0) / wprev)
                            nc.scalar.mul(
                                b3[:, m, :, :],
                                p3t.rearrange("p (two n) -> p two n", two=2),
                                b3_scale)

                            p2t = ppool.tile([128, 2 * nb], F32,
                                             name="p2", tag="p2")
                            mm_group(p2t[:, :], wt[2])
                            if const_sb is None:
                                sq2_scale = cons[m][:, C_SQ2 + dep:C_SQ2 + dep + 1]
                            else:
                                sq2_scale = float(
                                    np.sqrt(max(0.25 * const_sb[0], 0.0)) / wprev)
                            nc.scalar.activation(
                                sq2[:, m, :, :],
                                p2t.rearrange("p (two n) -> p two n", two=2),
                                AF.Square, scale=sq2_scale)

                            pWt = ppool.tile([128, 2 * nb], F32,
                                             name="pW", tag="pW")
                            mm_group(pWt[:, :], w1)
                            # pW evac carries the tanh affine: kw*s scale and
                            # the mixing bias, so tanh below is one plain op
                            if const_sb is None:
                                nc.scalar.activation(
                                    bw[:, m, :, :],
                                    pWt.rearrange("p (two n) -> p two n", two=2),
                                    AF.Identity,
                                    scale=cons[m][:, C_BW + dep:C_BW + dep + 1],
                                    bias=cons[m][:, C_BIAS:C_BIAS + 1])
                            elif const_sb[1] != 0.0:
                                nc.scalar.activation(
                                    bw[:, m, :, :],
                                    pWt.rearrange("p (two n) -> p two n", two=2),
                                    AF.Identity,
                                    scale=float(const_sb[0] / wprev),
                                    bias=float(const_sb[1]))
                            else:
                                nc.scalar.mul(
                                    bw[:, m, :, :],
                                    pWt.rearrange("p (two n) -> p two n", two=2),
                                    float(const_sb[0] / wprev))

                        # ---- batched chain ----
                        b3H, b3I = b3[:, :, 0, :], b3[:, :, 1, :]
                        accH, accI = acc[:, :, 0, :], acc[:, :, 1, :]
                        b5H, b5I = b5[:, :, 0, :], b5[:, :, 1, :]

                        nc.vector._custom_dve(R2OP, out=r5[:, :, :],
                                              in0=b5H, in1=b5I)
                        nc.scalar.activation(
                            s15[:, :, :], r5[:, :, :].bitcast(I32), AF.Exp,
                            scale=EXP_SCALE15,
                            bias=cons[0][:, C_B15 + dep:C_B15 + dep + 1])
                        nc.vector._custom_dve(CUBE3, out=accH, in0=b3H, in1=b3I)
                        nc.vector._custom_dve(CUBE3, out=accI, in0=b3I, in1=b3H)
                        nc.vector.tensor_tensor(b5H, b5H, s15[:, :, :],
                                                op=OP.mult)
                        nc.vector.tensor_tensor(b5I, b5I, s15[:, :, :],
                                                op=OP.mult)
                        nc.vector._custom_dve(QUINT, out=q5[:, :, 0, :],
                                              in0=b5H, in1=b5I, s0=QA, s1=QB)
                        nc.vector._custom_dve(QUINT, out=q5[:, :, 1, :],
                                              in0=b5I, in1=b5H, s0=QA, s1=QB)
                        nc.vector.tensor_tensor(acc[:, :, :, :], acc[:, :, :, :],
                                                q5[:, :, :, :], op=OP.add)
                        nc.vector.tensor_tensor(accH, accH, sq2[:, :, 0, :],
                                                op=OP.add)
                        nc.vector.tensor_tensor(accH, accH, sq2[:, :, 1, :],
                                                op=OP.add)
                        # last: the + (kw*s*pW + bias) term — only this add,
                        # tanh and sigma sit after the final evacuation
                        nc.vector.tensor_tensor(acc[:, :, :, :], acc[:, :, :, :],
                                                bw[:, :, :, :], op=OP.add)

                        nc.scalar.activation(snxt[:, :, :, :], acc[:, :, :, :],
                                             AF.Tanh)
                        for m in range(KT):
                            nc.vector.tensor_scalar_mul(
                                snxt[:, m, :, :], snxt[:, m, :, :],
                                cons[m][:, C_WF + dep:C_WF + dep + 1])
                        if dep < DEPTH - 1:
                            # fp8 copy of sigma for next depth's H5, split
                            # across ACT (H plane) and DVE (I plane) so
                            # neither engine eats the whole 4096-elem cast
                            nc.scalar.mul(f8nxt[:, :, 0, :], snxt[:, :, 0, :],
                                          SIG_SCALE)
                            nc.vector.tensor_scalar_mul(
                                f8nxt[:, :, 1, :], snxt[:, :, 1, :], SIG_SCALE)
                        # out accumulation on DVE (GpSimd would block DVE via
                        # the shared SBUF port pair)
                        if dep == 0:
                            # plain SBUF->SBUF DMA: idle engine, frees DVE
                            nc.sync.dma_start(out=out_t[:, :, :, :],
                                              in_=snxt[:, :, :, :])
                        else:
                            nc.vector.tensor_tensor(
                                out_t[:, :, :, :], out_t[:, :, :, :],
                                snxt[:, :, :, :], op=OP.add)

                for sl in range(2):
                    ci = 2 * cp + sl
                    c0 = ci * nb
                    for m in range(KT):
                        nc.sync.dma_start(
                            out=ore[m * 128:(m + 1) * 128, c0:c0 + nb],
                            in_=outs[sl][:, m, 0, :])
                        nc.sync.dma_start(
                            out=oim[m * 128:(m + 1) * 128, c0:c0 + nb],
                            in_=outs[sl][:, m, 1, :])
    nc.finalize()
    return nc


def host_prep(coupling_matrix, harmonic_1, harmonic_2, harmonic_3, harmonic_5,
              mixing_scale, mixing_bias):
    damping = (0.1 / (1.0 + np.exp(np.linspace(0.0, 3.0, D)))).astype(np.float64)
    fd = np.stack([np.exp(-damping * dd) for dd in range(DEPTH)])  # [7, D]
    wf = (WV[:, None] * fd).astype(np.float32)                     # [7, D]
    w1_0 = (coupling_matrix + harmonic_1).astype(np.float32)
    w1_r = w1_0 + np.eye(D, dtype=np.float32)
    wmat = np.ascontiguousarray(
        np.stack([w1_0, w1_r, harmonic_2, harmonic_3])
    ).astype(ml_dtypes.bfloat16)
    wm5 = np.ascontiguousarray(harmonic_5.astype(np.float32) * W5_SCALE
                               ).astype(ml_dtypes.float8_e4m3)
    s = mixing_scale.astype(np.float64)
    consts = np.zeros((D, NCONST), np.float32)
    consts[:, 0:DEPTH] = wf.T
    consts[:, C_BIAS] = mixing_bias.astype(np.float32)
    lns = np.log(np.maximum(s, 1e-30))
    for dep in range(DEPTH):
        wprev = 1.0 if dep == 0 else float(WV[dep - 1])
        b15 = (0.48 * LN2 * (127.0 - SIGMA0) + 0.2 * np.log(1.0 / 25.0)
               - 0.08 * np.log(wprev))
        consts[:, C_B15 + dep] = (b15 + 0.2 * lns).astype(np.float32)
        consts[:, C_BW + dep] = (s / wprev).astype(np.float32)
        consts[:, C_SQ2 + dep] = np.sqrt(np.maximum(0.25 * s, 0.0)
                                         ).astype(np.float32) / wprev
        consts[:, C_B3 + dep] = np.cbrt((1.0 / 9.0) * s).astype(np.float32) / wprev
    return wmat, wm5, consts


def device_inputs(wave_real, wave_imag, coupling_matrix, harmonic_1,
                  harmonic_2, harmonic_3, harmonic_5, mixing_scale,
                  mixing_bias):
    wmat, wm5, consts = host_prep(coupling_matrix, harmonic_1, harmonic_2,
                                  harmonic_3, harmonic_5, mixing_scale,
                                  mixing_bias)
    wreT = np.asarray(wave_real, np.float32).T
    wimT = np.asarray(wave_imag, np.float32).T
    wre_bf = wreT.astype(ml_dtypes.bfloat16)
    wim_bf = wimT.astype(ml_dtypes.bfloat16)
    wre_f8 = (wreT * SIG_SCALE).astype(ml_dtypes.float8_e4m3)
    wim_f8 = (wimT * SIG_SCALE).astype(ml_dtypes.float8_e4m3)
    in_maps = []
    for c in range(N_CORES):
        sl = slice(c * BS, (c + 1) * BS)
        in_maps.append({
            "wre": np.ascontiguousarray(wre_bf[:, sl]),
            "wim": np.ascontiguousarray(wim_bf[:, sl]),
            "wre8": np.ascontiguousarray(wre_f8[:, sl]),
            "wim8": np.ascontiguousarray(wim_f8[:, sl]),
            "wmat": wmat,
            "wm5": wm5,
            "consts": consts,
        })
    return in_maps


_NC_CACHE = {}


def _const_sb(mixing_scale, mixing_bias):
    s = np.asarray(mixing_scale, np.float32)
    b = np.asarray(mixing_bias, np.float32)
    if np.all(s == s.flat[0]) and np.all(b == b.flat[0]):
        return (float(s.flat[0]), float(b.flat[0]))
    return None


def _get_nc(n_chunks, nb, const_sb=None):
    key = (n_chunks, nb, const_sb)
    if key not in _NC_CACHE:
        _NC_CACHE[key] = build_program(n_chunks, nb, const_sb)
    return _NC_CACHE[key]


def kernel(wave_real, wave_imag, coupling_matrix, harmonic_1, harmonic_2,
           harmonic_3, harmonic_5, mixing_scale, mixing_bias):
    nc = _get_nc(BS // NB, NB, _const_sb(mixing_scale, mixing_bias))
    in_maps = device_inputs(wave_real, wave_imag, coupling_matrix, harmonic_1,
                            harmonic_2, harmonic_3, harmonic_5, mixing_scale,
                            mixing_bias)
    res = bass_utils.run_bass_kernel_spmd(nc, in_maps, core_ids=list(range(N_CORES)))
    out = np.empty((2, B, D), np.float32)
    for c in range(N_CORES):
        sl = slice(c * BS, (c + 1) * BS)
        out[0, sl, :] = res.results[c]["ore"].astype(np.float32).T
        out[1, sl, :] = res.results[c]["oim"].astype(np.float32).T
    return out
